# revision 2
# baseline (speedup 1.0000x reference)
"""Trainium2 Bass/Tile kernel for AttentionCombinerWithResidual (v2).

Reference computation (per batch element b):
    q = x_t @ Wq.T + bq ; k = x_s @ Wk.T + bk ; v = x_s @ Wv.T + bv
    w = softmax(q @ k.T / sqrt(D))
    out = layernorm(w @ v + x_t) * gamma + beta

Sharding: data-parallel over batch B=8 -> 8 NeuronCores, weights replicated,
no cross-core communication. Each core runs the full S=4096, D=256 attention.

v2 changes vs v1 (driven by HW microbenchmarks: ~60ns fixed overhead per
matmul with N>128 moving cols, ~75ns per tiny DVE op, scores+exp pipeline
runs at ACT rate):
  - epilogue: layernorm is scale-invariant, so instead of dividing combined
    by the softmax denominator, scale the residual:  LN(c/den + xt) ==
    LN(c + den*xt).  Removes the reciprocal + psum-scaled copy from the
    per-rowblock critical path (pc is freed by a single tensor_add).
  - epilogue: Newton-rsqrt for 1/sqrt(var) is batched [128, 8] per chunk
    (one op per Newton step for all 8 row-blocks) instead of [128, 1] ops
    per row-block; EPS dropped (var is den^2-scaled, eps contribution
    ~1e-5 relative -- far below tolerance).
  - x transpose psum->sbuf copies batched 4 s-tiles per instruction.
  - chunk-0 x_t DMA issued before the x_s stream so qT prep overlaps it.
  - scores loop kd-outer (stationary reuse order).
"""

import numpy as np
import ml_dtypes

B = 8
S = 4096
D = 256
P = 128
ND = D // P        # 2 d-tiles
NS = S // P        # 32 s-tiles
CH = 1024          # query-chunk width
NCH = S // CH      # 4 chunks
NQ = CH // 512     # 512-wide matmul slices per scores psum tile
NIB = CH // P      # 8 i-blocks per chunk
XB = 4             # s-tiles per batched x DMA
EPS = 1e-5
SCALE = 1.0 / 16.0  # 1/sqrt(D)

_CACHE = {}


def _build_nc(repeat=1, loop_n=0, stages=3, weave=0):
    """loop_n > 0 wraps the body in a device-side For_i loop (timing builds:
    the NEFF size stays constant while hardware work scales with loop_n).
    stages: 3=full kernel, 2=through combined matmul, 1=through scores/exp,
    0=input prep/projections only, 10=x_s load+transpose only."""
    import contextlib

    import concourse.bacc as bacc
    import concourse.bass as bass
    import concourse.tile as tile
    import concourse.mybir as mybir

    f32 = mybir.dt.float32
    bf16 = mybir.dt.bfloat16
    u32 = mybir.dt.uint32
    i32 = mybir.dt.int32
    AF = mybir.ActivationFunctionType
    OP = mybir.AluOpType

    nc = bacc.Bacc("TRN2", target_bir_lowering=False, debug=False)

    xt_d = nc.dram_tensor("xt", [S, D], f32, kind="ExternalInput")
    xs_d = nc.dram_tensor("xs", [S, D], f32, kind="ExternalInput")
    wq_d = nc.dram_tensor("wqt", [D, D], bf16, kind="ExternalInput")  # Wq.T
    wk_d = nc.dram_tensor("wkt", [D, D], bf16, kind="ExternalInput")
    wv_d = nc.dram_tensor("wvt", [D, D], bf16, kind="ExternalInput")
    bq_d = nc.dram_tensor("bq", [D], f32, kind="ExternalInput")
    bk_d = nc.dram_tensor("bk", [D], f32, kind="ExternalInput")
    bv_d = nc.dram_tensor("bv", [D], f32, kind="ExternalInput")
    g_d = nc.dram_tensor("gamma", [D], f32, kind="ExternalInput")
    be_d = nc.dram_tensor("beta", [D], f32, kind="ExternalInput")
    out_d = nc.dram_tensor("out", [S, D], f32, kind="ExternalOutput")

    def bcast(dram_ap, n):
        return bass.AP(
            tensor=dram_ap.tensor, offset=dram_ap.offset, ap=[[0, n]] + list(dram_ap.ap)
        )

    xt_batches = xt_d.ap().rearrange("(b t p) c -> b p t c", t=XB, p=P)
    xs_batches = xs_d.ap().rearrange("(b t p) c -> b p t c", t=XB, p=P)
    out_rows = out_d.ap().rearrange("(t p) c -> t p c", p=P)

    with tile.TileContext(nc) as tc:
        with (
            tc.tile_pool(name="persist", bufs=1) as persist,
            tc.tile_pool(name="xsload", bufs=3) as xsload,
            tc.tile_pool(name="xtload", bufs=5) as xtload,
            tc.tile_pool(name="wpool", bufs=34) as wpool,
            tc.tile_pool(name="xtprep", bufs=2) as xtprep,
            tc.tile_pool(name="epi", bufs=12) as epi,
            tc.tile_pool(name="stats", bufs=6) as stats,
            tc.tile_pool(name="psA", bufs=3, space="PSUM") as psA,
            tc.tile_pool(name="psB", bufs=2, space="PSUM") as psB,
        ):
            from concourse.masks import make_identity

            ident = persist.tile([P, P], f32)

            # chunk-0 x_t batches first in the DMA queue: the first PE work
            # (transposes) waits on these
            xt0_tiles = []
            for half in range(CH // (XB * P)):
                xn = xtload.tile([P, XB, D], f32, tag="xn")
                nc.sync.dma_start(xn[:], xt_batches[half])
                xt0_tiles.append(xn)

            make_identity(nc, ident)

            # replicated constants
            wqs = persist.tile([P, ND, D], bf16)
            wks = persist.tile([P, ND, D], bf16)
            wvs = persist.tile([P, ND, D], bf16)
            nc.sync.dma_start(wqs[:], wq_d.ap().rearrange("(t p) c -> p t c", p=P))
            nc.sync.dma_start(wks[:], wk_d.ap().rearrange("(t p) c -> p t c", p=P))
            nc.sync.dma_start(wvs[:], wv_d.ap().rearrange("(t p) c -> p t c", p=P))
            bq_sb = persist.tile([P, ND], f32)
            bk_sb = persist.tile([P, ND], f32)
            nc.sync.dma_start(bq_sb[:], bq_d.ap().rearrange("(t p) -> p t", p=P))
            nc.sync.dma_start(bk_sb[:], bk_d.ap().rearrange("(t p) -> p t", p=P))
            bv_bc = persist.tile([P, D], f32)
            gm_bc = persist.tile([P, D], f32)
            bt_bc = persist.tile([P, D], f32)
            nc.sync.dma_start(bv_bc[:], bcast(bv_d.ap(), P))
            nc.sync.dma_start(gm_bc[:], bcast(g_d.ap(), P))
            nc.sync.dma_start(bt_bc[:], bcast(be_d.ap(), P))

            xsT = persist.tile([P, ND, S], bf16)  # [p, kd, s] = x_s[s, kd*P+p]
            kT = persist.tile([P, ND, S], bf16)   # [p, mo, s] = k[s, mo*P+p]
            v_sb = persist.tile([P, NS, D + 1], bf16)  # [p, jt, c]; c==D is ones

            def consume_tiles(slices):
                ps = psB.tile([P, 1], f32, tag="ps_small")
                for idx, sl in enumerate(slices):
                    nc.tensor.matmul(
                        ps[:],
                        sl[:, 0:P],
                        sl[:, 0:1],
                        start=(idx == 0),
                        stop=(idx == len(slices) - 1),
                    )
                dst = stats.tile([P, 1], f32, tag="consume")
                nc.vector.tensor_copy(dst[:], ps[:])
                nc.sync.dma_start(out_rows[0][:, 0:1], dst[:])

            def load_transpose(batches, bt, dstT, col0, pool, eng):
                """DMA XB s-tiles of x (rows); PE-transpose into dstT columns
                starting at col0 (one batched psum->sbuf copy)."""
                xn = pool.tile([P, XB, D], f32, tag="xn")
                nc.sync.dma_start(xn[:], batches[bt])
                pst = psA.tile([P, ND, XB * P], f32, tag="ps_sc")
                for t in range(XB):
                    for kd in range(ND):
                        nc.tensor.transpose(
                            pst[:, kd, t * P : (t + 1) * P],
                            xn[:, t, kd * P : (kd + 1) * P],
                            ident[:],
                        )
                if eng is nc.scalar:
                    nc.scalar.copy(dstT[:, :, col0 : col0 + XB * P], pst[:])
                else:
                    eng.tensor_copy(dstT[:, :, col0 : col0 + XB * P], pst[:])
                return xn

            def project(wsb, xT, ncols, dstT, bias_sb, use_act):
                """dstT[:, mo, 0:ncols] = wsb.T @ xT + bias, 512 cols/psum."""
                for mo in range(ND):
                    for s0 in range(0, ncols, 512):
                        ps = psA.tile([P, 512], f32, tag="ps_sc")
                        for kd in range(ND):
                            nc.tensor.matmul(
                                ps[:],
                                wsb[:, kd, mo * P : (mo + 1) * P],
                                xT[:, kd, s0 : s0 + 512],
                                start=(kd == 0),
                                stop=(kd == ND - 1),
                            )
                        if use_act:
                            nc.scalar.activation(
                                dstT[:, mo, s0 : s0 + 512],
                                ps[:],
                                AF.Identity,
                                bias=bias_sb[:, mo : mo + 1],
                            )
                        else:
                            nc.vector.tensor_scalar_add(
                                dstT[:, mo, s0 : s0 + 512],
                                ps[:],
                                bias_sb[:, mo : mo + 1],
                            )

            def prep_load(c):
                """x_t DMAs for chunk c (trace early: streams behind compute)."""
                tiles = []
                for half in range(CH // (XB * P)):  # 2 batched loads per chunk
                    bt = c * (CH // (XB * P)) + half
                    xn = xtload.tile([P, XB, D], f32, tag="xn")
                    nc.sync.dma_start(xn[:], xt_batches[bt])
                    tiles.append(xn)
                return tiles

            def prep_compute(c, tiles):
                """Transpose + qT projection for chunk c's loaded x_t."""
                xtTc = xtprep.tile([P, ND, CH], bf16, tag="xtT")
                qTc = xtprep.tile([P, ND, CH], bf16, tag="qT")
                for half, xn in enumerate(tiles):
                    col0 = half * XB * P
                    pst = psA.tile([P, ND, XB * P], f32, tag="ps_sc")
                    for t in range(XB):
                        for kd in range(ND):
                            nc.tensor.transpose(
                                pst[:, kd, t * P : (t + 1) * P],
                                xn[:, t, kd * P : (kd + 1) * P],
                                ident[:],
                            )
                    nc.vector.tensor_copy(xtTc[:, :, col0 : col0 + XB * P], pst[:])
                project(wqs, xtTc, CH, qTc, bq_sb, use_act=False)
                return tiles, qTc

            def prep_chunk(c):
                return prep_compute(c, prep_load(c))

            def epilogue_block(pc, ib, xn_res, den_b, mv_b, z_tiles):
                """Per row-block: scale-invariant residual add + LN stats.
                z' = combined + den * xt ; stats stashed for the batched tail."""
                nc.vector.tensor_copy(den_b[:, ib : ib + 1], pc[:, D : D + 1])
                zt = epi.tile([P, D], f32, tag="z")
                nc.vector.tensor_scalar_mul(zt[:], xn_res, den_b[:, ib : ib + 1])
                nc.vector.tensor_add(zt[:], zt[:], pc[:, 0:D])  # frees pc
                st6 = stats.tile([P, 6], f32, tag="st6")
                nc.vector.bn_stats(st6[:], zt[:])
                nc.vector.bn_aggr(mv_b[:, 2 * ib : 2 * ib + 2], st6[:])
                z_tiles.append(zt)

            def epilogue_tail(c, mv_b, z_tiles, ib0, nib):
                """Batched 1/sqrt(var) for row-blocks [ib0, ib0+nib), then
                final normalize + affine + store per block."""
                var = mv_b[:, 2 * ib0 + 1 : 2 * (ib0 + nib) : 2]  # [P, nib]
                y = stats.tile([P, nib], f32, tag="y")
                yi = y.bitcast(u32)
                a = stats.tile([P, nib], f32, tag="a")
                nc.vector.tensor_copy(a[:], var)
                nc.vector.tensor_scalar(
                    yi[:], a.bitcast(u32)[:], 1, None, op0=OP.logical_shift_right
                )
                nc.vector.tensor_scalar(
                    yi[:], yi[:], 0xFFFFFFFF, None, op0=OP.bitwise_xor
                )
                yi_s = y.bitcast(i32)
                nc.vector.tensor_scalar(
                    yi_s[:], yi_s[:], 0x5F3759E0, None, op0=OP.add
                )
                u = stats.tile([P, nib], f32, tag="u")
                for _ in range(2):
                    nc.vector.tensor_mul(u[:], y[:], y[:])
                    nc.vector.tensor_mul(u[:], u[:], a[:])
                    nc.vector.tensor_scalar(
                        u[:], u[:], -0.5, 1.5, op0=OP.mult, op1=OP.add
                    )
                    nc.vector.tensor_mul(y[:], y[:], u[:])
                for i in range(nib):
                    ib = ib0 + i
                    gi = c * NIB + ib
                    o = epi.tile([P, D], f32, tag="o")
                    nc.vector.tensor_scalar(
                        o[:],
                        z_tiles[ib][:],
                        mv_b[:, 2 * ib : 2 * ib + 1],
                        y[:, i : i + 1],
                        op0=OP.subtract,
                        op1=OP.mult,
                    )
                    nc.gpsimd.tensor_mul(o[:], o[:], gm_bc[:])
                    nc.gpsimd.tensor_add(o[:], o[:], bt_bc[:])
                    nc.sync.dma_start(out_rows[gi], o[:])

            def score_tile(qTc, jt):
                """scoresT psum tile for key-tile jt vs the chunk's queries,
                exp'd into a bf16 w tile."""
                ps = psA.tile([P, CH], f32, tag="ps_sc")
                for kd in range(ND):
                    for q in range(NQ):
                        nc.tensor.matmul(
                            ps[:, q * 512 : (q + 1) * 512],
                            kT[:, kd, jt * P : (jt + 1) * P],
                            qTc[:, kd, q * 512 : (q + 1) * 512],
                            start=(kd == 0),
                            stop=(kd == ND - 1),
                        )
                wt = wpool.tile([P, CH], bf16, tag="w")
                nc.scalar.activation(wt[:], ps[:], AF.Exp, scale=SCALE)
                return wt

            def body():
                # chunk-0 x_t was DMA'd before everything else; its PE-side
                # prep overlaps the x_s DMA stream
                prep = {0: prep_compute(0, xt0_tiles)}

                # ---- phase A: x_s side, software-pipelined: each XB-batch's
                # transposes, kT slice, v slice, and chunk-0 scores for the
                # batch's 4 key-tiles run while later DMAs stream in. ACT is
                # kept exp-only (copies and bias adds on DVE). ----
                nc.vector.memset(v_sb[:, :, D : D + 1], 1.0)
                w_tiles0 = []
                for bt in range(NS // XB):
                    load_transpose(
                        xs_batches,
                        bt,
                        xsT,
                        bt * XB * P,
                        xsload,
                        nc.scalar if bt % 2 == 0 else nc.vector,
                    )
                    if stages == 10:
                        continue
                    s0 = bt * XB * P  # 512 kT columns per batch
                    for mo in range(ND):
                        ps = psA.tile([P, 512], f32, tag="ps_sc")
                        for kd in range(ND):
                            nc.tensor.matmul(
                                ps[:],
                                wks[:, kd, mo * P : (mo + 1) * P],
                                xsT[:, kd, s0 : s0 + 512],
                                start=(kd == 0),
                                stop=(kd == ND - 1),
                            )
                        nc.scalar.activation(
                            kT[:, mo, s0 : s0 + 512],
                            ps[:],
                            AF.Identity,
                            bias=bk_sb[:, mo : mo + 1],
                        )
                    for st in range(bt * XB, (bt + 1) * XB):
                        ps = psA.tile([P, D], f32, tag="ps_sc")
                        for kd in range(ND):
                            nc.tensor.matmul(
                                ps[:],
                                xsT[:, kd, st * P : (st + 1) * P],
                                wvs[:, kd, :],
                                start=(kd == 0),
                                stop=(kd == ND - 1),
                            )
                        nc.vector.tensor_add(v_sb[:, st, 0:D], ps[:], bv_bc[:])
                    if stages >= 1 and weave:
                        for jt in range(bt * XB, (bt + 1) * XB):
                            w_tiles0.append(score_tile(prep[0][1], jt))

                if stages == 10:
                    consume_tiles([xsT[:, mo, :] for mo in range(ND)])
                    return

                if stages == 0:
                    consume_tiles(
                        [prep[0][1][:, mo, :] for mo in range(ND)]
                        + [kT[:, mo, :] for mo in range(ND)]
                        + [v_sb[:, jt, :] for jt in range(NS)]
                    )
                    return

                # ---- main loop: attention per query chunk ----
                for c in range(NCH):
                    qTc = prep[c][1]
                    if c == 0 and weave:
                        w_tiles = w_tiles0
                    else:
                        w_tiles = [score_tile(qTc, jt) for jt in range(NS)]

                    # next chunk's x_t DMA streams during this chunk's
                    # combined phase; its PE work is traced after (below)
                    if c + 1 < NCH:
                        next_tiles = prep_load(c + 1)

                    if stages == 1:
                        consume_tiles(w_tiles)
                        if c + 1 < NCH:
                            prep[c + 1] = prep_compute(c + 1, next_tiles)
                        continue

                    den_b = stats.tile([P, NIB], f32, tag="den")
                    mv_b = stats.tile([P, 2 * NIB], f32, tag="mv")
                    z_tiles = []
                    for ib in range(NIB):
                        pc = psB.tile([P, D + 1], f32, tag="ps_small")
                        for jt in range(NS):
                            nc.tensor.matmul(
                                pc[:],
                                w_tiles[jt][:, ib * P : (ib + 1) * P],
                                v_sb[:, jt, :],
                                start=(jt == 0),
                                stop=(jt == NS - 1),
                            )
                        if stages == 2:
                            dst = epi.tile([P, D + 1], f32, tag="z")
                            nc.vector.tensor_copy(dst[:], pc[:])
                            nc.sync.dma_start(
                                out_rows[c * NIB + ib][:, 0:1], dst[:, 0:1]
                            )
                            continue
                        xn_res = prep[c][0][ib // XB][:, ib % XB, :]
                        epilogue_block(pc, ib, xn_res, den_b, mv_b, z_tiles)
                        # first half-batch of the tail as soon as blocks 0-3
                        # are in (shortens the end-of-chunk drain)
                        if stages != 2 and ib == NIB // 2 - 1:
                            epilogue_tail(c, mv_b, z_tiles, 0, NIB // 2)
                    # next chunk's PE-side prep precedes the epilogue tail so
                    # its qT is ready before scores(c+1) (tail is DVE/Pool)
                    if c + 1 < NCH:
                        prep[c + 1] = prep_compute(c + 1, next_tiles)
                    if stages != 2:
                        epilogue_tail(c, mv_b, z_tiles, NIB // 2, NIB // 2)

            loop_cm = (
                tc.For_i(0, loop_n, 1) if loop_n > 0 else contextlib.nullcontext()
            )
            with loop_cm:
                for _rep in range(repeat):
                    body()

    nc.compile()
    return nc


def _get_nc(repeat=1, loop_n=0, stages=3, weave=0):
    key = ("nc", repeat, loop_n, stages, weave)
    if key not in _CACHE:
        _CACHE[key] = _build_nc(repeat, loop_n, stages, weave)
    return _CACHE[key]


def _make_in_maps(
    supervised_embedding,
    transformer_embedding,
    Wq,
    bq,
    Wk,
    bk,
    Wv,
    bv,
    gamma,
    beta,
):
    bf = ml_dtypes.bfloat16
    f32 = np.float32
    shared = {
        "wqt": np.ascontiguousarray(np.asarray(Wq, f32).T).astype(bf),
        "wkt": np.ascontiguousarray(np.asarray(Wk, f32).T).astype(bf),
        "wvt": np.ascontiguousarray(np.asarray(Wv, f32).T).astype(bf),
        "bq": np.ascontiguousarray(np.asarray(bq, f32)),
        "bk": np.ascontiguousarray(np.asarray(bk, f32)),
        "bv": np.ascontiguousarray(np.asarray(bv, f32)),
        "gamma": np.ascontiguousarray(np.asarray(gamma, f32)),
        "beta": np.ascontiguousarray(np.asarray(beta, f32)),
    }
    xs_all = np.asarray(supervised_embedding, f32)
    xt_all = np.asarray(transformer_embedding, f32)
    return [
        {
            "xt": np.ascontiguousarray(xt_all[b]),
            "xs": np.ascontiguousarray(xs_all[b]),
            **shared,
        }
        for b in range(B)
    ]


def kernel(**inputs):
    from concourse.bass_utils import run_bass_kernel_spmd

    nc = _get_nc()
    in_maps = _make_in_maps(**inputs)
    res = run_bass_kernel_spmd(nc, in_maps, core_ids=list(range(B)))
    return np.stack([res.results[b]["out"] for b in range(B)], axis=0)


# revision 3
# speedup vs baseline: 1.1587x; 1.1587x over previous
"""Trainium2 Bass/Tile kernel for AttentionCombinerWithResidual (v2).

Reference computation (per batch element b):
    q = x_t @ Wq.T + bq ; k = x_s @ Wk.T + bk ; v = x_s @ Wv.T + bv
    w = softmax(q @ k.T / sqrt(D))
    out = layernorm(w @ v + x_t) * gamma + beta

Sharding: data-parallel over batch B=8 -> 8 NeuronCores, weights replicated,
no cross-core communication. Each core runs the full S=4096, D=256 attention.

v2 changes vs v1 (driven by HW microbenchmarks: ~60ns fixed overhead per
matmul with N>128 moving cols, ~75ns per tiny DVE op, scores+exp pipeline
runs at ACT rate):
  - epilogue: layernorm is scale-invariant, so instead of dividing combined
    by the softmax denominator, scale the residual:  LN(c/den + xt) ==
    LN(c + den*xt).  Removes the reciprocal + psum-scaled copy from the
    per-rowblock critical path (pc is freed by a single tensor_add).
  - epilogue: Newton-rsqrt for 1/sqrt(var) is batched [128, 8] per chunk
    (one op per Newton step for all 8 row-blocks) instead of [128, 1] ops
    per row-block; EPS dropped (var is den^2-scaled, eps contribution
    ~1e-5 relative -- far below tolerance).
  - x transpose psum->sbuf copies batched 4 s-tiles per instruction.
  - chunk-0 x_t DMA issued before the x_s stream so qT prep overlaps it.
  - scores loop kd-outer (stationary reuse order).
"""

import numpy as np
import ml_dtypes

B = 8
S = 4096
D = 256
P = 128
ND = D // P        # 2 d-tiles
NS = S // P        # 32 s-tiles
CH = 1024          # query-chunk width
NCH = S // CH      # 4 chunks
NQ = CH // 512     # 512-wide matmul slices per scores psum tile
NIB = CH // P      # 8 i-blocks per chunk
XB = 4             # s-tiles per batched x DMA
EPS = 1e-5
SCALE = 1.0 / 16.0  # 1/sqrt(D)

_CACHE = {}


def _build_nc(repeat=1, loop_n=0, stages=3, weave=0, fp8=0):
    """loop_n > 0 wraps the body in a device-side For_i loop (timing builds:
    the NEFF size stays constant while hardware work scales with loop_n).
    stages: 3=full kernel, 2=through combined matmul, 1=through scores/exp,
    0=input prep/projections only, 10=x_s load+transpose only."""
    import contextlib

    import concourse.bacc as bacc
    import concourse.bass as bass
    import concourse.tile as tile
    import concourse.mybir as mybir

    f32 = mybir.dt.float32
    bf16 = mybir.dt.bfloat16
    f8 = mybir.dt.float8e4
    qk_dt = f8 if fp8 else bf16
    u32 = mybir.dt.uint32
    i32 = mybir.dt.int32
    AF = mybir.ActivationFunctionType
    OP = mybir.AluOpType

    nc = bacc.Bacc("TRN2", target_bir_lowering=False, debug=False)

    xt_d = nc.dram_tensor("xt", [S, D], f32, kind="ExternalInput")
    xst_d = nc.dram_tensor("xst", [P, ND, S], bf16, kind="ExternalInput")
    xtt_d = nc.dram_tensor("xtt", [P, ND, S], bf16, kind="ExternalInput")
    wq_d = nc.dram_tensor("wqt", [D, D], bf16, kind="ExternalInput")  # Wq.T
    wk_d = nc.dram_tensor("wkt", [D, D], bf16, kind="ExternalInput")
    wv_d = nc.dram_tensor("wvt", [D, D], bf16, kind="ExternalInput")
    bq_d = nc.dram_tensor("bq", [D], f32, kind="ExternalInput")
    bk_d = nc.dram_tensor("bk", [D], f32, kind="ExternalInput")
    bv_d = nc.dram_tensor("bv", [D], f32, kind="ExternalInput")
    g_d = nc.dram_tensor("gamma", [D], f32, kind="ExternalInput")
    be_d = nc.dram_tensor("beta", [D], f32, kind="ExternalInput")
    out_d = nc.dram_tensor("out", [S, D], f32, kind="ExternalOutput")

    def bcast(dram_ap, n):
        return bass.AP(
            tensor=dram_ap.tensor, offset=dram_ap.offset, ap=[[0, n]] + list(dram_ap.ap)
        )

    xt_batches = xt_d.ap().rearrange("(b t p) c -> b p t c", t=XB, p=P)
    out_rows = out_d.ap().rearrange("(t p) c -> t p c", p=P)

    with tile.TileContext(nc) as tc:
        with (
            tc.tile_pool(name="persist", bufs=1) as persist,
            tc.tile_pool(name="xsload", bufs=3) as xsload,
            tc.tile_pool(name="xtload", bufs=5) as xtload,
            tc.tile_pool(name="wpool", bufs=34) as wpool,
            tc.tile_pool(name="xtprep", bufs=2) as xtprep,
            tc.tile_pool(name="epi", bufs=12) as epi,
            tc.tile_pool(name="stats", bufs=6) as stats,
            tc.tile_pool(name="psA", bufs=3, space="PSUM") as psA,
            tc.tile_pool(name="psB", bufs=2, space="PSUM") as psB,
        ):
            # chunk-0 x_t batches first in the DMA queue: the first PE work
            # (qT projection) waits on these
            xt0_tiles = []
            for half in range(CH // (XB * P)):
                xn = xtload.tile([P, XB, D], f32, tag="xn")
                nc.sync.dma_start(xn[:], xt_batches[half])
                xt0_tiles.append(xn)
            xtT0 = xtprep.tile([P, ND, CH], bf16, tag="xtT")
            nc.sync.dma_start(xtT0[:], xtt_d.ap()[:, :, 0:CH])

            # replicated constants
            wqs = persist.tile([P, ND, D], bf16)
            wks = persist.tile([P, ND, D], bf16)
            wvs = persist.tile([P, ND, D], bf16)
            nc.sync.dma_start(wqs[:], wq_d.ap().rearrange("(t p) c -> p t c", p=P))
            nc.sync.dma_start(wks[:], wk_d.ap().rearrange("(t p) c -> p t c", p=P))
            nc.sync.dma_start(wvs[:], wv_d.ap().rearrange("(t p) c -> p t c", p=P))
            bq_sb = persist.tile([P, ND], f32)
            bk_sb = persist.tile([P, ND], f32)
            nc.sync.dma_start(bq_sb[:], bq_d.ap().rearrange("(t p) -> p t", p=P))
            nc.sync.dma_start(bk_sb[:], bk_d.ap().rearrange("(t p) -> p t", p=P))
            bv_bc = persist.tile([P, D], f32)
            gm_bc = persist.tile([P, D], f32)
            bt_bc = persist.tile([P, D], f32)
            nc.sync.dma_start(bv_bc[:], bcast(bv_d.ap(), P))
            nc.sync.dma_start(gm_bc[:], bcast(g_d.ap(), P))
            nc.sync.dma_start(bt_bc[:], bcast(be_d.ap(), P))

            xsT = persist.tile([P, ND, S], bf16)  # [p, kd, s] = x_s[s, kd*P+p]
            kT = persist.tile([P, ND, S], qk_dt)  # [p, mo, s] = k[s, mo*P+p]
            v_sb = persist.tile([P, NS, D + 1], bf16)  # [p, jt, c]; c==D is ones

            def consume_tiles(slices):
                ps = psB.tile([P, 1], f32, tag="ps_small")
                for idx, sl in enumerate(slices):
                    nc.tensor.matmul(
                        ps[:],
                        sl[:, 0:P],
                        sl[:, 0:1],
                        start=(idx == 0),
                        stop=(idx == len(slices) - 1),
                    )
                dst = stats.tile([P, 1], f32, tag="consume")
                nc.vector.tensor_copy(dst[:], ps[:])
                nc.sync.dma_start(out_rows[0][:, 0:1], dst[:])

            def project(wsb, xT, ncols, dstT, bias_sb, use_act):
                """dstT[:, mo, 0:ncols] = wsb.T @ xT + bias, 512 cols/psum."""
                for mo in range(ND):
                    for s0 in range(0, ncols, 512):
                        ps = psA.tile([P, 512], f32, tag="ps_sc")
                        for kd in range(ND):
                            nc.tensor.matmul(
                                ps[:],
                                wsb[:, kd, mo * P : (mo + 1) * P],
                                xT[:, kd, s0 : s0 + 512],
                                start=(kd == 0),
                                stop=(kd == ND - 1),
                            )
                        if use_act:
                            nc.scalar.activation(
                                dstT[:, mo, s0 : s0 + 512],
                                ps[:],
                                AF.Identity,
                                bias=bias_sb[:, mo : mo + 1],
                            )
                        else:
                            nc.vector.tensor_scalar_add(
                                dstT[:, mo, s0 : s0 + 512],
                                ps[:],
                                bias_sb[:, mo : mo + 1],
                            )

            def prep_load(c):
                """x_t DMAs for chunk c: residual row batches + the
                host-pre-transposed xtT slice (trace early)."""
                tiles = []
                for half in range(CH // (XB * P)):  # 2 batched loads per chunk
                    bt = c * (CH // (XB * P)) + half
                    xn = xtload.tile([P, XB, D], f32, tag="xn")
                    nc.sync.dma_start(xn[:], xt_batches[bt])
                    tiles.append(xn)
                xtTc = xtprep.tile([P, ND, CH], bf16, tag="xtT")
                nc.sync.dma_start(
                    xtTc[:], xtt_d.ap()[:, :, c * CH : (c + 1) * CH]
                )
                return tiles, xtTc

            def prep_compute(c, loaded):
                """qT projection for chunk c's loaded xtT."""
                tiles, xtTc = loaded
                qTc = xtprep.tile([P, ND, CH], qk_dt, tag="qT")
                project(wqs, xtTc, CH, qTc, bq_sb, use_act=False)
                return tiles, qTc

            def prep_chunk(c):
                return prep_compute(c, prep_load(c))

            def epilogue_block(pc, ib, xn_res, den_b, mv_b, z_tiles):
                """Per row-block: scale-invariant residual add + LN stats.
                z' = combined + den * xt ; stats stashed for the batched tail."""
                nc.vector.tensor_copy(den_b[:, ib : ib + 1], pc[:, D : D + 1])
                zt = epi.tile([P, D], f32, tag="z")
                nc.vector.tensor_scalar_mul(zt[:], xn_res, den_b[:, ib : ib + 1])
                nc.vector.tensor_add(zt[:], zt[:], pc[:, 0:D])  # frees pc
                st6 = stats.tile([P, 6], f32, tag="st6")
                nc.vector.bn_stats(st6[:], zt[:])
                nc.vector.bn_aggr(mv_b[:, 2 * ib : 2 * ib + 2], st6[:])
                z_tiles.append(zt)

            def epilogue_tail(c, mv_b, z_tiles, ib0, nib):
                """Batched 1/sqrt(var) for row-blocks [ib0, ib0+nib), then
                final normalize + affine + store per block."""
                var = mv_b[:, 2 * ib0 + 1 : 2 * (ib0 + nib) : 2]  # [P, nib]
                y = stats.tile([P, nib], f32, tag="y")
                yi = y.bitcast(u32)
                a = stats.tile([P, nib], f32, tag="a")
                nc.vector.tensor_copy(a[:], var)
                nc.vector.tensor_scalar(
                    yi[:], a.bitcast(u32)[:], 1, None, op0=OP.logical_shift_right
                )
                nc.vector.tensor_scalar(
                    yi[:], yi[:], 0xFFFFFFFF, None, op0=OP.bitwise_xor
                )
                yi_s = y.bitcast(i32)
                nc.vector.tensor_scalar(
                    yi_s[:], yi_s[:], 0x5F3759E0, None, op0=OP.add
                )
                u = stats.tile([P, nib], f32, tag="u")
                for _ in range(2):
                    nc.vector.tensor_mul(u[:], y[:], y[:])
                    nc.vector.tensor_mul(u[:], u[:], a[:])
                    nc.vector.tensor_scalar(
                        u[:], u[:], -0.5, 1.5, op0=OP.mult, op1=OP.add
                    )
                    nc.vector.tensor_mul(y[:], y[:], u[:])
                for i in range(nib):
                    ib = ib0 + i
                    gi = c * NIB + ib
                    o = epi.tile([P, D], f32, tag="o")
                    nc.vector.tensor_scalar(
                        o[:],
                        z_tiles[ib][:],
                        mv_b[:, 2 * ib : 2 * ib + 1],
                        y[:, i : i + 1],
                        op0=OP.subtract,
                        op1=OP.mult,
                    )
                    nc.gpsimd.tensor_mul(o[:], o[:], gm_bc[:])
                    nc.gpsimd.tensor_add(o[:], o[:], bt_bc[:])
                    nc.sync.dma_start(out_rows[gi], o[:])

            def score_tile(qTc, jt):
                """scoresT psum tile for key-tile jt vs the chunk's queries,
                exp'd into a bf16 w tile."""
                ps = psA.tile([P, CH], f32, tag="ps_sc")
                if fp8:
                    # DoubleRow: both d-halves contracted in one MM
                    # (lhsT [Ki, 2, M], rhs [Ki, 2, N])
                    for q in range(NQ):
                        nc.tensor.matmul(
                            ps[:, q * 512 : (q + 1) * 512],
                            kT[:, :, jt * P : (jt + 1) * P],
                            qTc[:, :, q * 512 : (q + 1) * 512],
                            start=True,
                            stop=True,
                            perf_mode=mybir.MatmulPerfMode.DoubleRow,
                        )
                else:
                    for kd in range(ND):
                        for q in range(NQ):
                            nc.tensor.matmul(
                                ps[:, q * 512 : (q + 1) * 512],
                                kT[:, kd, jt * P : (jt + 1) * P],
                                qTc[:, kd, q * 512 : (q + 1) * 512],
                                start=(kd == 0),
                                stop=(kd == ND - 1),
                            )
                wt = wpool.tile([P, CH], bf16, tag="w")
                nc.scalar.activation(wt[:], ps[:], AF.Exp, scale=SCALE)
                return wt

            def body():
                # chunk-0 x_t was DMA'd before everything else; its PE-side
                # prep overlaps the x_s DMA stream
                prep = {0: prep_compute(0, (xt0_tiles, xtT0))}

                # ---- phase A: x_s side, software-pipelined: each XB-batch's
                # transposes, kT slice, v slice, and chunk-0 scores for the
                # batch's 4 key-tiles run while later DMAs stream in. ACT is
                # kept exp-only (copies and bias adds on DVE). ----
                nc.vector.memset(v_sb[:, :, D : D + 1], 1.0)
                w_tiles0 = []
                for bt in range(NS // XB):
                    s0 = bt * XB * P  # 512 columns per batch
                    # host-pre-transposed x_s slice streams straight into xsT
                    nc.sync.dma_start(
                        xsT[:, :, s0 : s0 + XB * P],
                        xst_d.ap()[:, :, s0 : s0 + XB * P],
                    )
                    if stages == 10:
                        continue
                    for mo in range(ND):
                        ps = psA.tile([P, 512], f32, tag="ps_sc")
                        for kd in range(ND):
                            nc.tensor.matmul(
                                ps[:],
                                wks[:, kd, mo * P : (mo + 1) * P],
                                xsT[:, kd, s0 : s0 + 512],
                                start=(kd == 0),
                                stop=(kd == ND - 1),
                            )
                        nc.scalar.activation(
                            kT[:, mo, s0 : s0 + 512],
                            ps[:],
                            AF.Identity,
                            bias=bk_sb[:, mo : mo + 1],
                        )
                    for st in range(bt * XB, (bt + 1) * XB):
                        ps = psA.tile([P, D], f32, tag="ps_sc")
                        for kd in range(ND):
                            nc.tensor.matmul(
                                ps[:],
                                xsT[:, kd, st * P : (st + 1) * P],
                                wvs[:, kd, :],
                                start=(kd == 0),
                                stop=(kd == ND - 1),
                            )
                        nc.vector.tensor_add(v_sb[:, st, 0:D], ps[:], bv_bc[:])
                    if stages >= 1 and weave:
                        for jt in range(bt * XB, (bt + 1) * XB):
                            w_tiles0.append(score_tile(prep[0][1], jt))

                if stages == 10:
                    consume_tiles([xsT[:, mo, :] for mo in range(ND)])
                    return

                if stages == 0:
                    consume_tiles(
                        [prep[0][1][:, mo, :] for mo in range(ND)]
                        + [kT[:, mo, :] for mo in range(ND)]
                        + [v_sb[:, jt, :] for jt in range(NS)]
                    )
                    return

                # ---- main loop: attention per query chunk ----
                for c in range(NCH):
                    qTc = prep[c][1]
                    if c == 0 and weave:
                        w_tiles = w_tiles0
                    else:
                        w_tiles = [score_tile(qTc, jt) for jt in range(NS)]

                    # next chunk's x_t DMA streams during this chunk's
                    # combined phase; its PE work is traced after (below)
                    if c + 1 < NCH:
                        next_tiles = prep_load(c + 1)

                    if stages == 1:
                        consume_tiles(w_tiles)
                        if c + 1 < NCH:
                            prep[c + 1] = prep_compute(c + 1, next_tiles)
                        continue

                    den_b = stats.tile([P, NIB], f32, tag="den")
                    mv_b = stats.tile([P, 2 * NIB], f32, tag="mv")
                    z_tiles = []
                    for ib in range(NIB):
                        pc = psB.tile([P, D + 1], f32, tag="ps_small")
                        for jt in range(NS):
                            nc.tensor.matmul(
                                pc[:],
                                w_tiles[jt][:, ib * P : (ib + 1) * P],
                                v_sb[:, jt, :],
                                start=(jt == 0),
                                stop=(jt == NS - 1),
                            )
                        if stages == 2:
                            dst = epi.tile([P, D + 1], f32, tag="z")
                            nc.vector.tensor_copy(dst[:], pc[:])
                            nc.sync.dma_start(
                                out_rows[c * NIB + ib][:, 0:1], dst[:, 0:1]
                            )
                            continue
                        xn_res = prep[c][0][ib // XB][:, ib % XB, :]
                        epilogue_block(pc, ib, xn_res, den_b, mv_b, z_tiles)
                        # first half-batch of the tail as soon as blocks 0-3
                        # are in (shortens the end-of-chunk drain)
                        if stages != 2 and ib == NIB // 2 - 1:
                            epilogue_tail(c, mv_b, z_tiles, 0, NIB // 2)
                    # next chunk's PE-side prep precedes the epilogue tail so
                    # its qT is ready before scores(c+1) (tail is DVE/Pool)
                    if c + 1 < NCH:
                        prep[c + 1] = prep_compute(c + 1, next_tiles)
                    if stages != 2:
                        epilogue_tail(c, mv_b, z_tiles, NIB // 2, NIB // 2)

            loop_cm = (
                tc.For_i(0, loop_n, 1) if loop_n > 0 else contextlib.nullcontext()
            )
            with loop_cm:
                for _rep in range(repeat):
                    body()

    nc.compile()
    return nc


def _get_nc(repeat=1, loop_n=0, stages=3, weave=0, fp8=0):
    key = ("nc", repeat, loop_n, stages, weave, fp8)
    if key not in _CACHE:
        _CACHE[key] = _build_nc(repeat, loop_n, stages, weave, fp8)
    return _CACHE[key]


def _make_in_maps(
    supervised_embedding,
    transformer_embedding,
    Wq,
    bq,
    Wk,
    bk,
    Wv,
    bv,
    gamma,
    beta,
):
    bf = ml_dtypes.bfloat16
    f32 = np.float32
    shared = {
        "wqt": np.ascontiguousarray(np.asarray(Wq, f32).T).astype(bf),
        "wkt": np.ascontiguousarray(np.asarray(Wk, f32).T).astype(bf),
        "wvt": np.ascontiguousarray(np.asarray(Wv, f32).T).astype(bf),
        "bq": np.ascontiguousarray(np.asarray(bq, f32)),
        "bk": np.ascontiguousarray(np.asarray(bk, f32)),
        "bv": np.ascontiguousarray(np.asarray(bv, f32)),
        "gamma": np.ascontiguousarray(np.asarray(gamma, f32)),
        "beta": np.ascontiguousarray(np.asarray(beta, f32)),
    }
    xs_all = np.asarray(supervised_embedding, f32)
    xt_all = np.asarray(transformer_embedding, f32)

    def xT(x):
        # [S, D] rows -> [P, ND, S] with xT[p, kd, s] = x[s, kd*P + p]
        return np.ascontiguousarray(
            x.reshape(4096, 2, 128).transpose(2, 1, 0)
        ).astype(bf)

    return [
        {
            "xt": np.ascontiguousarray(xt_all[b]),
            "xst": xT(xs_all[b]),
            "xtt": xT(xt_all[b]),
            **shared,
        }
        for b in range(B)
    ]


def kernel(**inputs):
    from concourse.bass_utils import run_bass_kernel_spmd

    nc = _get_nc()
    in_maps = _make_in_maps(**inputs)
    res = run_bass_kernel_spmd(nc, in_maps, core_ids=list(range(B)))
    return np.stack([res.results[b]["out"] for b in range(B)], axis=0)


# revision 4
# speedup vs baseline: 1.2199x; 1.0528x over previous
"""Trainium2 Bass/Tile kernel for AttentionCombinerWithResidual (v2).

Reference computation (per batch element b):
    q = x_t @ Wq.T + bq ; k = x_s @ Wk.T + bk ; v = x_s @ Wv.T + bv
    w = softmax(q @ k.T / sqrt(D))
    out = layernorm(w @ v + x_t) * gamma + beta

Sharding: data-parallel over batch B=8 -> 8 NeuronCores, weights replicated,
no cross-core communication. Each core runs the full S=4096, D=256 attention.

v2 changes vs v1 (driven by HW microbenchmarks: ~60ns fixed overhead per
matmul with N>128 moving cols, ~75ns per tiny DVE op, scores+exp pipeline
runs at ACT rate):
  - epilogue: layernorm is scale-invariant, so instead of dividing combined
    by the softmax denominator, scale the residual:  LN(c/den + xt) ==
    LN(c + den*xt).  Removes the reciprocal + psum-scaled copy from the
    per-rowblock critical path (pc is freed by a single tensor_add).
  - epilogue: Newton-rsqrt for 1/sqrt(var) is batched [128, 8] per chunk
    (one op per Newton step for all 8 row-blocks) instead of [128, 1] ops
    per row-block; EPS dropped (var is den^2-scaled, eps contribution
    ~1e-5 relative -- far below tolerance).
  - x transpose psum->sbuf copies batched 4 s-tiles per instruction.
  - chunk-0 x_t DMA issued before the x_s stream so qT prep overlaps it.
  - scores loop kd-outer (stationary reuse order).
"""

import numpy as np
import ml_dtypes

B = 8
S = 4096
D = 256
P = 128
ND = D // P        # 2 d-tiles
NS = S // P        # 32 s-tiles
CH = 1024          # query-chunk width
NCH = S // CH      # 4 chunks
NQ = CH // 512     # 512-wide matmul slices per scores psum tile
NIB = CH // P      # 8 i-blocks per chunk
XB = 4             # s-tiles per batched x DMA
EPS = 1e-5
SCALE = 1.0 / 16.0  # 1/sqrt(D)

_CACHE = {}


def _build_nc(repeat=1, loop_n=0, stages=3, weave=0, fp8=0):
    """loop_n > 0 wraps the body in a device-side For_i loop (timing builds:
    the NEFF size stays constant while hardware work scales with loop_n).
    stages: 3=full kernel, 2=through combined matmul, 1=through scores/exp,
    0=input prep/projections only, 10=x_s load+transpose only."""
    import contextlib

    import concourse.bacc as bacc
    import concourse.bass as bass
    import concourse.tile as tile
    import concourse.mybir as mybir

    f32 = mybir.dt.float32
    bf16 = mybir.dt.bfloat16
    f8 = mybir.dt.float8e4
    qk_dt = f8 if fp8 else bf16
    u32 = mybir.dt.uint32
    i32 = mybir.dt.int32
    AF = mybir.ActivationFunctionType
    OP = mybir.AluOpType

    nc = bacc.Bacc("TRN2", target_bir_lowering=False, debug=False)

    xt_d = nc.dram_tensor("xt", [S, D], f32, kind="ExternalInput")
    xst_d = nc.dram_tensor("xst", [P, ND, S], bf16, kind="ExternalInput")
    xtt_d = nc.dram_tensor("xtt", [P, ND, S], bf16, kind="ExternalInput")
    wq_d = nc.dram_tensor("wqt", [D, D], bf16, kind="ExternalInput")  # Wq.T
    wk_d = nc.dram_tensor("wkt", [D, D], bf16, kind="ExternalInput")
    wv_d = nc.dram_tensor("wvt", [D, D], bf16, kind="ExternalInput")
    bq_d = nc.dram_tensor("bq", [D], f32, kind="ExternalInput")
    bk_d = nc.dram_tensor("bk", [D], f32, kind="ExternalInput")
    bv_d = nc.dram_tensor("bv", [D], f32, kind="ExternalInput")
    g_d = nc.dram_tensor("gamma", [D], f32, kind="ExternalInput")
    be_d = nc.dram_tensor("beta", [D], f32, kind="ExternalInput")
    out_d = nc.dram_tensor("out", [S, D], f32, kind="ExternalOutput")

    def bcast(dram_ap, n):
        return bass.AP(
            tensor=dram_ap.tensor, offset=dram_ap.offset, ap=[[0, n]] + list(dram_ap.ap)
        )

    xt_batches = xt_d.ap().rearrange("(b t p) c -> b p t c", t=XB, p=P)
    out_rows = out_d.ap().rearrange("(t p) c -> t p c", p=P)

    with tile.TileContext(nc) as tc:
        with (
            tc.tile_pool(name="persist", bufs=1) as persist,
            tc.tile_pool(name="xsload", bufs=3) as xsload,
            tc.tile_pool(name="xtload", bufs=5) as xtload,
            tc.tile_pool(name="wpool", bufs=34) as wpool,
            tc.tile_pool(name="xtprep", bufs=2) as xtprep,
            tc.tile_pool(name="epi", bufs=12) as epi,
            tc.tile_pool(name="stats", bufs=6) as stats,
            tc.tile_pool(name="psA", bufs=3, space="PSUM") as psA,
            tc.tile_pool(name="psB", bufs=2, space="PSUM") as psB,
        ):
            # DMA queue order: the first PE work is the chunk-0 qT
            # projection -- it needs only wqs and xtT0, so those go first;
            # the residual row batches follow
            wqs = persist.tile([P, ND, D], bf16)
            nc.sync.dma_start(wqs[:], wq_d.ap().rearrange("(t p) c -> p t c", p=P))
            xtT0 = xtprep.tile([P, ND, CH], bf16, tag="xtT")
            nc.sync.dma_start(xtT0[:, :, 0:512], xtt_d.ap()[:, :, 0:512])
            nc.sync.dma_start(xtT0[:, :, 512:CH], xtt_d.ap()[:, :, 512:CH])
            xt0_tiles = []
            for half in range(CH // (XB * P)):
                xn = xtload.tile([P, XB, D], f32, tag="xn")
                nc.sync.dma_start(xn[:], xt_batches[half])
                xt0_tiles.append(xn)

            # replicated constants
            wks = persist.tile([P, ND, D], bf16)
            wvs = persist.tile([P, ND, D], bf16)
            nc.sync.dma_start(wks[:], wk_d.ap().rearrange("(t p) c -> p t c", p=P))
            nc.sync.dma_start(wvs[:], wv_d.ap().rearrange("(t p) c -> p t c", p=P))
            bq_sb = persist.tile([P, ND], f32)
            bk_sb = persist.tile([P, ND], f32)
            nc.sync.dma_start(bq_sb[:], bq_d.ap().rearrange("(t p) -> p t", p=P))
            nc.sync.dma_start(bk_sb[:], bk_d.ap().rearrange("(t p) -> p t", p=P))
            bv_bc = persist.tile([P, D], f32)
            gm_bc = persist.tile([P, D], f32)
            bt_bc = persist.tile([P, D], f32)
            nc.sync.dma_start(bv_bc[:], bcast(bv_d.ap(), P))
            nc.sync.dma_start(gm_bc[:], bcast(g_d.ap(), P))
            nc.sync.dma_start(bt_bc[:], bcast(be_d.ap(), P))

            xsT = persist.tile([P, ND, S], bf16)  # [p, kd, s] = x_s[s, kd*P+p]
            kT = persist.tile([P, ND, S], qk_dt)  # [p, mo, s] = k[s, mo*P+p]
            v_sb = persist.tile([P, NS, D + 1], bf16)  # [p, jt, c]; c==D is ones

            def consume_tiles(slices):
                ps = psB.tile([P, 1], f32, tag="ps_small")
                for idx, sl in enumerate(slices):
                    nc.tensor.matmul(
                        ps[:],
                        sl[:, 0:P],
                        sl[:, 0:1],
                        start=(idx == 0),
                        stop=(idx == len(slices) - 1),
                    )
                dst = stats.tile([P, 1], f32, tag="consume")
                nc.vector.tensor_copy(dst[:], ps[:])
                nc.sync.dma_start(out_rows[0][:, 0:1], dst[:])

            def project(wsb, xT, ncols, dstT, bias_sb, use_act):
                """dstT[:, mo, 0:ncols] = wsb.T @ xT + bias, 512 cols/psum."""
                for mo in range(ND):
                    for s0 in range(0, ncols, 512):
                        ps = psA.tile([P, 512], f32, tag="ps_sc")
                        for kd in range(ND):
                            nc.tensor.matmul(
                                ps[:],
                                wsb[:, kd, mo * P : (mo + 1) * P],
                                xT[:, kd, s0 : s0 + 512],
                                start=(kd == 0),
                                stop=(kd == ND - 1),
                            )
                        if use_act:
                            nc.scalar.activation(
                                dstT[:, mo, s0 : s0 + 512],
                                ps[:],
                                AF.Identity,
                                bias=bias_sb[:, mo : mo + 1],
                            )
                        else:
                            nc.vector.tensor_scalar_add(
                                dstT[:, mo, s0 : s0 + 512],
                                ps[:],
                                bias_sb[:, mo : mo + 1],
                            )

            def prep_load(c):
                """x_t DMAs for chunk c: residual row batches + the
                host-pre-transposed xtT slice (trace early)."""
                tiles = []
                for half in range(CH // (XB * P)):  # 2 batched loads per chunk
                    bt = c * (CH // (XB * P)) + half
                    xn = xtload.tile([P, XB, D], f32, tag="xn")
                    nc.sync.dma_start(xn[:], xt_batches[bt])
                    tiles.append(xn)
                xtTc = xtprep.tile([P, ND, CH], bf16, tag="xtT")
                nc.sync.dma_start(
                    xtTc[:], xtt_d.ap()[:, :, c * CH : (c + 1) * CH]
                )
                return tiles, xtTc

            def prep_compute(c, loaded):
                """qT projection for chunk c's loaded xtT."""
                tiles, xtTc = loaded
                qTc = xtprep.tile([P, ND, CH], qk_dt, tag="qT")
                project(wqs, xtTc, CH, qTc, bq_sb, use_act=False)
                return tiles, qTc

            def prep_chunk(c):
                return prep_compute(c, prep_load(c))

            def epilogue_block(pc, ib, xn_res, den_b, mv_b, z_tiles):
                """Per row-block: scale-invariant residual add + LN stats.
                z' = combined + den * xt ; stats stashed for the batched tail."""
                nc.vector.tensor_copy(den_b[:, ib : ib + 1], pc[:, D : D + 1])
                zt = epi.tile([P, D], f32, tag="z")
                nc.vector.tensor_scalar_mul(zt[:], xn_res, den_b[:, ib : ib + 1])
                nc.vector.tensor_add(zt[:], zt[:], pc[:, 0:D])  # frees pc
                st6 = stats.tile([P, 6], f32, tag="st6")
                nc.vector.bn_stats(st6[:], zt[:])
                nc.vector.bn_aggr(mv_b[:, 2 * ib : 2 * ib + 2], st6[:])
                z_tiles.append(zt)

            def epilogue_tail(c, mv_b, z_tiles, ib0, nib):
                """Batched 1/sqrt(var) for row-blocks [ib0, ib0+nib), then
                final normalize + affine + store per block."""
                var = mv_b[:, 2 * ib0 + 1 : 2 * (ib0 + nib) : 2]  # [P, nib]
                y = stats.tile([P, nib], f32, tag="y")
                yi = y.bitcast(u32)
                a = stats.tile([P, nib], f32, tag="a")
                nc.vector.tensor_copy(a[:], var)
                nc.vector.tensor_scalar(
                    yi[:], a.bitcast(u32)[:], 1, None, op0=OP.logical_shift_right
                )
                nc.vector.tensor_scalar(
                    yi[:], yi[:], 0xFFFFFFFF, None, op0=OP.bitwise_xor
                )
                yi_s = y.bitcast(i32)
                nc.vector.tensor_scalar(
                    yi_s[:], yi_s[:], 0x5F3759E0, None, op0=OP.add
                )
                u = stats.tile([P, nib], f32, tag="u")
                for _ in range(2):
                    nc.vector.tensor_mul(u[:], y[:], y[:])
                    nc.vector.tensor_mul(u[:], u[:], a[:])
                    nc.vector.tensor_scalar(
                        u[:], u[:], -0.5, 1.5, op0=OP.mult, op1=OP.add
                    )
                    nc.vector.tensor_mul(y[:], y[:], u[:])
                for i in range(nib):
                    ib = ib0 + i
                    gi = c * NIB + ib
                    o = epi.tile([P, D], f32, tag="o")
                    nc.vector.tensor_scalar(
                        o[:],
                        z_tiles[ib][:],
                        mv_b[:, 2 * ib : 2 * ib + 1],
                        y[:, i : i + 1],
                        op0=OP.subtract,
                        op1=OP.mult,
                    )
                    nc.gpsimd.tensor_mul(o[:], o[:], gm_bc[:])
                    nc.gpsimd.tensor_add(o[:], o[:], bt_bc[:])
                    nc.sync.dma_start(out_rows[gi], o[:])

            def score_tile(qTc, jt):
                """scoresT psum tile for key-tile jt vs the chunk's queries,
                exp'd into a bf16 w tile."""
                ps = psA.tile([P, CH], f32, tag="ps_sc")
                if fp8:
                    # DoubleRow: both d-halves contracted in one MM
                    # (lhsT [Ki, 2, M], rhs [Ki, 2, N])
                    for q in range(NQ):
                        nc.tensor.matmul(
                            ps[:, q * 512 : (q + 1) * 512],
                            kT[:, :, jt * P : (jt + 1) * P],
                            qTc[:, :, q * 512 : (q + 1) * 512],
                            start=True,
                            stop=True,
                            perf_mode=mybir.MatmulPerfMode.DoubleRow,
                        )
                else:
                    for kd in range(ND):
                        for q in range(NQ):
                            nc.tensor.matmul(
                                ps[:, q * 512 : (q + 1) * 512],
                                kT[:, kd, jt * P : (jt + 1) * P],
                                qTc[:, kd, q * 512 : (q + 1) * 512],
                                start=(kd == 0),
                                stop=(kd == ND - 1),
                            )
                wt = wpool.tile([P, CH], bf16, tag="w")
                nc.scalar.activation(wt[:], ps[:], AF.Exp, scale=SCALE)
                return wt

            def body():
                # chunk-0 x_t was DMA'd before everything else; its PE-side
                # prep overlaps the x_s DMA stream
                prep = {0: prep_compute(0, (xt0_tiles, xtT0))}

                # ---- phase A: x_s side, software-pipelined: each XB-batch's
                # transposes, kT slice, v slice, and chunk-0 scores for the
                # batch's 4 key-tiles run while later DMAs stream in. ACT is
                # kept exp-only (copies and bias adds on DVE). ----
                nc.vector.memset(v_sb[:, :, D : D + 1], 1.0)
                w_tiles0 = []
                for bt in range(NS // XB):
                    s0 = bt * XB * P  # 512 columns per batch
                    # host-pre-transposed x_s slice streams straight into xsT
                    nc.sync.dma_start(
                        xsT[:, :, s0 : s0 + XB * P],
                        xst_d.ap()[:, :, s0 : s0 + XB * P],
                    )
                    if stages == 10:
                        continue
                    for mo in range(ND):
                        ps = psA.tile([P, 512], f32, tag="ps_sc")
                        for kd in range(ND):
                            nc.tensor.matmul(
                                ps[:],
                                wks[:, kd, mo * P : (mo + 1) * P],
                                xsT[:, kd, s0 : s0 + 512],
                                start=(kd == 0),
                                stop=(kd == ND - 1),
                            )
                        nc.scalar.activation(
                            kT[:, mo, s0 : s0 + 512],
                            ps[:],
                            AF.Identity,
                            bias=bk_sb[:, mo : mo + 1],
                        )
                    for st in range(bt * XB, (bt + 1) * XB):
                        ps = psA.tile([P, D], f32, tag="ps_sc")
                        for kd in range(ND):
                            nc.tensor.matmul(
                                ps[:],
                                xsT[:, kd, st * P : (st + 1) * P],
                                wvs[:, kd, :],
                                start=(kd == 0),
                                stop=(kd == ND - 1),
                            )
                        nc.vector.tensor_add(v_sb[:, st, 0:D], ps[:], bv_bc[:])
                    if stages >= 1 and weave:
                        for jt in range(bt * XB, (bt + 1) * XB):
                            w_tiles0.append(score_tile(prep[0][1], jt))

                if stages == 10:
                    consume_tiles([xsT[:, mo, :] for mo in range(ND)])
                    return

                if stages == 0:
                    consume_tiles(
                        [prep[0][1][:, mo, :] for mo in range(ND)]
                        + [kT[:, mo, :] for mo in range(ND)]
                        + [v_sb[:, jt, :] for jt in range(NS)]
                    )
                    return

                # ---- main loop: attention per query chunk ----
                for c in range(NCH):
                    qTc = prep[c][1]
                    if c == 0 and weave:
                        w_tiles = w_tiles0
                    else:
                        w_tiles = [score_tile(qTc, jt) for jt in range(NS)]

                    # next chunk's x_t DMA streams during this chunk's
                    # combined phase; its PE work is traced after (below)
                    if c + 1 < NCH:
                        next_tiles = prep_load(c + 1)

                    if stages == 1:
                        consume_tiles(w_tiles)
                        if c + 1 < NCH:
                            prep[c + 1] = prep_compute(c + 1, next_tiles)
                        continue

                    den_b = stats.tile([P, NIB], f32, tag="den")
                    mv_b = stats.tile([P, 2 * NIB], f32, tag="mv")
                    z_tiles = []
                    for ib in range(NIB):
                        pc = psB.tile([P, D + 1], f32, tag="ps_small")
                        for jt in range(NS):
                            nc.tensor.matmul(
                                pc[:],
                                w_tiles[jt][:, ib * P : (ib + 1) * P],
                                v_sb[:, jt, :],
                                start=(jt == 0),
                                stop=(jt == NS - 1),
                            )
                        if stages == 2:
                            dst = epi.tile([P, D + 1], f32, tag="z")
                            nc.vector.tensor_copy(dst[:], pc[:])
                            nc.sync.dma_start(
                                out_rows[c * NIB + ib][:, 0:1], dst[:, 0:1]
                            )
                            continue
                        xn_res = prep[c][0][ib // XB][:, ib % XB, :]
                        epilogue_block(pc, ib, xn_res, den_b, mv_b, z_tiles)
                        # drain the tail early: halves normally, pairs on
                        # the final chunk (shortens the post-matmul drain)
                        if stages != 2:
                            if c == NCH - 1 and ib % 2 == 1 and ib < NIB - 1:
                                epilogue_tail(c, mv_b, z_tiles, ib - 1, 2)
                            elif c < NCH - 1 and ib == NIB // 2 - 1:
                                epilogue_tail(c, mv_b, z_tiles, 0, NIB // 2)
                    # next chunk's PE-side prep precedes the epilogue tail so
                    # its qT is ready before scores(c+1) (tail is DVE/Pool)
                    if c + 1 < NCH:
                        prep[c + 1] = prep_compute(c + 1, next_tiles)
                    if stages != 2:
                        if c == NCH - 1:
                            epilogue_tail(c, mv_b, z_tiles, NIB - 2, 2)
                        else:
                            epilogue_tail(c, mv_b, z_tiles, NIB // 2, NIB // 2)

            loop_cm = (
                tc.For_i(0, loop_n, 1) if loop_n > 0 else contextlib.nullcontext()
            )
            with loop_cm:
                for _rep in range(repeat):
                    body()

    nc.compile()
    return nc


def _get_nc(repeat=1, loop_n=0, stages=3, weave=0, fp8=0):
    key = ("nc", repeat, loop_n, stages, weave, fp8)
    if key not in _CACHE:
        _CACHE[key] = _build_nc(repeat, loop_n, stages, weave, fp8)
    return _CACHE[key]


def _make_in_maps(
    supervised_embedding,
    transformer_embedding,
    Wq,
    bq,
    Wk,
    bk,
    Wv,
    bv,
    gamma,
    beta,
):
    bf = ml_dtypes.bfloat16
    f32 = np.float32
    shared = {
        "wqt": np.ascontiguousarray(np.asarray(Wq, f32).T).astype(bf),
        "wkt": np.ascontiguousarray(np.asarray(Wk, f32).T).astype(bf),
        "wvt": np.ascontiguousarray(np.asarray(Wv, f32).T).astype(bf),
        "bq": np.ascontiguousarray(np.asarray(bq, f32)),
        "bk": np.ascontiguousarray(np.asarray(bk, f32)),
        "bv": np.ascontiguousarray(np.asarray(bv, f32)),
        "gamma": np.ascontiguousarray(np.asarray(gamma, f32)),
        "beta": np.ascontiguousarray(np.asarray(beta, f32)),
    }
    xs_all = np.asarray(supervised_embedding, f32)
    xt_all = np.asarray(transformer_embedding, f32)

    def xT(x):
        # [S, D] rows -> [P, ND, S] with xT[p, kd, s] = x[s, kd*P + p]
        return np.ascontiguousarray(
            x.reshape(4096, 2, 128).transpose(2, 1, 0)
        ).astype(bf)

    return [
        {
            "xt": np.ascontiguousarray(xt_all[b]),
            "xst": xT(xs_all[b]),
            "xtt": xT(xt_all[b]),
            **shared,
        }
        for b in range(B)
    ]


def kernel(**inputs):
    from concourse.bass_utils import run_bass_kernel_spmd

    nc = _get_nc()
    in_maps = _make_in_maps(**inputs)
    res = run_bass_kernel_spmd(nc, in_maps, core_ids=list(range(B)))
    return np.stack([res.results[b]["out"] for b in range(B)], axis=0)


# revision 5
# speedup vs baseline: 1.5285x; 1.2530x over previous
"""Trainium2 Bass/Tile kernel for AttentionCombinerWithResidual (v2).

Reference computation (per batch element b):
    q = x_t @ Wq.T + bq ; k = x_s @ Wk.T + bk ; v = x_s @ Wv.T + bv
    w = softmax(q @ k.T / sqrt(D))
    out = layernorm(w @ v + x_t) * gamma + beta

Sharding: data-parallel over batch B=8 -> 8 NeuronCores, weights replicated,
no cross-core communication. Each core runs the full S=4096, D=256 attention.

v2 changes vs v1 (driven by HW microbenchmarks: ~60ns fixed overhead per
matmul with N>128 moving cols, ~75ns per tiny DVE op, scores+exp pipeline
runs at ACT rate):
  - epilogue: layernorm is scale-invariant, so instead of dividing combined
    by the softmax denominator, scale the residual:  LN(c/den + xt) ==
    LN(c + den*xt).  Removes the reciprocal + psum-scaled copy from the
    per-rowblock critical path (pc is freed by a single tensor_add).
  - epilogue: Newton-rsqrt for 1/sqrt(var) is batched [128, 8] per chunk
    (one op per Newton step for all 8 row-blocks) instead of [128, 1] ops
    per row-block; EPS dropped (var is den^2-scaled, eps contribution
    ~1e-5 relative -- far below tolerance).
  - x transpose psum->sbuf copies batched 4 s-tiles per instruction.
  - chunk-0 x_t DMA issued before the x_s stream so qT prep overlaps it.
  - scores loop kd-outer (stationary reuse order).
"""

import numpy as np
import ml_dtypes

B = 8
S = 4096
D = 256
P = 128
ND = D // P        # 2 d-tiles
NS = S // P        # 32 s-tiles
CH = 1024          # query-chunk width
NCH = S // CH      # 4 chunks
NQ = CH // 512     # 512-wide matmul slices per scores psum tile
NIB = CH // P      # 8 i-blocks per chunk
XB = 4             # s-tiles per batched x DMA
EPS = 1e-5
SCALE = 1.0 / 16.0  # 1/sqrt(D)

_CACHE = {}


def _build_nc(repeat=1, loop_n=0, stages=3, weave=0, fp8=0):
    """loop_n > 0 wraps the body in a device-side For_i loop (timing builds:
    the NEFF size stays constant while hardware work scales with loop_n).
    stages: 3=full kernel, 2=through combined matmul, 1=through scores/exp,
    0=input prep/projections only, 10=x_s load+transpose only."""
    import contextlib

    import concourse.bacc as bacc
    import concourse.bass as bass
    import concourse.tile as tile
    import concourse.mybir as mybir

    f32 = mybir.dt.float32
    bf16 = mybir.dt.bfloat16
    f8 = mybir.dt.float8e4
    qk_dt = f8 if fp8 else bf16
    u32 = mybir.dt.uint32
    i32 = mybir.dt.int32
    AF = mybir.ActivationFunctionType
    OP = mybir.AluOpType

    nc = bacc.Bacc("TRN2", target_bir_lowering=False, debug=False)

    xt_d = nc.dram_tensor("xt", [S, D], f32, kind="ExternalInput")
    xst_d = nc.dram_tensor("xst", [P, ND, S], bf16, kind="ExternalInput")
    xtt_d = nc.dram_tensor("xtt", [P, ND, S], bf16, kind="ExternalInput")
    wq_d = nc.dram_tensor("wqt", [D, D], bf16, kind="ExternalInput")  # Wq.T
    wk_d = nc.dram_tensor("wkt", [D, D], bf16, kind="ExternalInput")
    wv_d = nc.dram_tensor("wvt", [D, D], bf16, kind="ExternalInput")
    bq_d = nc.dram_tensor("bq", [D], f32, kind="ExternalInput")
    bk_d = nc.dram_tensor("bk", [D], f32, kind="ExternalInput")
    bv_d = nc.dram_tensor("bv", [D], f32, kind="ExternalInput")
    g_d = nc.dram_tensor("gamma", [D], f32, kind="ExternalInput")
    be_d = nc.dram_tensor("beta", [D], f32, kind="ExternalInput")
    out_d = nc.dram_tensor("out", [S, D], f32, kind="ExternalOutput")

    def bcast(dram_ap, n):
        return bass.AP(
            tensor=dram_ap.tensor, offset=dram_ap.offset, ap=[[0, n]] + list(dram_ap.ap)
        )

    xt_batches = xt_d.ap().rearrange("(b t p) c -> b p t c", t=XB, p=P)
    out_rows = out_d.ap().rearrange("(t p) c -> t p c", p=P)

    with tile.TileContext(nc) as tc:
        with (
            tc.tile_pool(name="persist", bufs=1) as persist,
            tc.tile_pool(name="xsload", bufs=3) as xsload,
            tc.tile_pool(name="xtload", bufs=5) as xtload,
            tc.tile_pool(name="wpool", bufs=34) as wpool,
            tc.tile_pool(name="xtprep", bufs=2) as xtprep,
            tc.tile_pool(name="epi", bufs=12) as epi,
            tc.tile_pool(name="stats", bufs=6) as stats,
            tc.tile_pool(name="psA", bufs=3, space="PSUM") as psA,
            tc.tile_pool(name="psB", bufs=2, space="PSUM") as psB,
        ):
            # DMA queue order: the first PE work is the chunk-0 qT
            # projection -- it needs only wqs and xtT0, so those go first;
            # the residual row batches follow
            wqs = persist.tile([P, ND, D], bf16)
            nc.sync.dma_start(wqs[:], wq_d.ap().rearrange("(t p) c -> p t c", p=P))
            xtT0 = xtprep.tile([P, ND, CH], bf16, tag="xtT")
            nc.sync.dma_start(xtT0[:, :, 0:512], xtt_d.ap()[:, :, 0:512])
            nc.sync.dma_start(xtT0[:, :, 512:CH], xtt_d.ap()[:, :, 512:CH])
            xt0_tiles = []
            for half in range(CH // (XB * P)):
                xn = xtload.tile([P, XB, D], f32, tag="xn")
                nc.sync.dma_start(xn[:], xt_batches[half])
                xt0_tiles.append(xn)

            # replicated constants
            wks = persist.tile([P, ND, D], bf16)
            wvs = persist.tile([P, ND, D], bf16)
            nc.sync.dma_start(wks[:], wk_d.ap().rearrange("(t p) c -> p t c", p=P))
            nc.sync.dma_start(wvs[:], wv_d.ap().rearrange("(t p) c -> p t c", p=P))
            bq_sb = persist.tile([P, ND], f32)
            bk_sb = persist.tile([P, ND], f32)
            nc.sync.dma_start(bq_sb[:], bq_d.ap().rearrange("(t p) -> p t", p=P))
            nc.sync.dma_start(bk_sb[:], bk_d.ap().rearrange("(t p) -> p t", p=P))
            bv_bc = persist.tile([P, D], f32)
            gm_bc = persist.tile([P, D], f32)
            bt_bc = persist.tile([P, D], f32)
            nc.sync.dma_start(bv_bc[:], bcast(bv_d.ap(), P))
            nc.sync.dma_start(gm_bc[:], bcast(g_d.ap(), P))
            nc.sync.dma_start(bt_bc[:], bcast(be_d.ap(), P))

            xsT = persist.tile([P, ND, S], bf16)  # [p, kd, s] = x_s[s, kd*P+p]
            kT = persist.tile([P, ND, S], qk_dt)  # [p, mo, s] = k[s, mo*P+p]
            v_sb = persist.tile([P, NS, D + 1], bf16)  # [p, jt, c]; c==D is ones

            def consume_tiles(slices):
                ps = psB.tile([P, 1], f32, tag="ps_small")
                for idx, sl in enumerate(slices):
                    nc.tensor.matmul(
                        ps[:],
                        sl[:, 0:P],
                        sl[:, 0:1],
                        start=(idx == 0),
                        stop=(idx == len(slices) - 1),
                    )
                dst = stats.tile([P, 1], f32, tag="consume")
                nc.vector.tensor_copy(dst[:], ps[:])
                nc.sync.dma_start(out_rows[0][:, 0:1], dst[:])

            def project(wsb, xT, ncols, dstT, bias_sb, use_act):
                """dstT[:, mo, 0:ncols] = wsb.T @ xT + bias, 512 cols/psum."""
                for mo in range(ND):
                    for s0 in range(0, ncols, 512):
                        ps = psA.tile([P, 512], f32, tag="ps_sc")
                        for kd in range(ND):
                            nc.tensor.matmul(
                                ps[:],
                                wsb[:, kd, mo * P : (mo + 1) * P],
                                xT[:, kd, s0 : s0 + 512],
                                start=(kd == 0),
                                stop=(kd == ND - 1),
                            )
                        if use_act:
                            nc.scalar.activation(
                                dstT[:, mo, s0 : s0 + 512],
                                ps[:],
                                AF.Identity,
                                bias=bias_sb[:, mo : mo + 1],
                            )
                        else:
                            nc.vector.tensor_scalar_add(
                                dstT[:, mo, s0 : s0 + 512],
                                ps[:],
                                bias_sb[:, mo : mo + 1],
                            )

            def prep_load(c):
                """x_t DMAs for chunk c: residual row batches + the
                host-pre-transposed xtT slice (trace early)."""
                tiles = []
                for half in range(CH // (XB * P)):  # 2 batched loads per chunk
                    bt = c * (CH // (XB * P)) + half
                    xn = xtload.tile([P, XB, D], f32, tag="xn")
                    nc.sync.dma_start(xn[:], xt_batches[bt])
                    tiles.append(xn)
                xtTc = xtprep.tile([P, ND, CH], bf16, tag="xtT")
                nc.sync.dma_start(
                    xtTc[:], xtt_d.ap()[:, :, c * CH : (c + 1) * CH]
                )
                return tiles, xtTc

            def prep_compute(c, loaded):
                """qT projection for chunk c's loaded xtT."""
                tiles, xtTc = loaded
                qTc = xtprep.tile([P, ND, CH], qk_dt, tag="qT")
                project(wqs, xtTc, CH, qTc, bq_sb, use_act=False)
                return tiles, qTc

            def prep_chunk(c):
                return prep_compute(c, prep_load(c))

            def epilogue_block(pc, ib, xn_res, den_b, mv_b, z_tiles):
                """Per row-block: scale-invariant residual add + LN stats.
                z' = combined + den * xt ; stats stashed for the batched tail."""
                nc.vector.tensor_copy(den_b[:, ib : ib + 1], pc[:, D : D + 1])
                zt = epi.tile([P, D], f32, tag="z")
                nc.vector.tensor_scalar_mul(zt[:], xn_res, den_b[:, ib : ib + 1])
                nc.vector.tensor_add(zt[:], zt[:], pc[:, 0:D])  # frees pc
                st6 = stats.tile([P, 6], f32, tag="st6")
                nc.vector.bn_stats(st6[:], zt[:])
                nc.vector.bn_aggr(mv_b[:, 2 * ib : 2 * ib + 2], st6[:])
                z_tiles.append(zt)

            def epilogue_tail(c, mv_b, z_tiles, ib0, nib):
                """Batched 1/sqrt(var) for row-blocks [ib0, ib0+nib), then
                final normalize + affine + store per block."""
                var = mv_b[:, 2 * ib0 + 1 : 2 * (ib0 + nib) : 2]  # [P, nib]
                y = stats.tile([P, nib], f32, tag="y")
                yi = y.bitcast(u32)
                a = stats.tile([P, nib], f32, tag="a")
                nc.vector.tensor_copy(a[:], var)
                nc.vector.tensor_scalar(
                    yi[:], a.bitcast(u32)[:], 1, None, op0=OP.logical_shift_right
                )
                nc.vector.tensor_scalar(
                    yi[:], yi[:], 0xFFFFFFFF, None, op0=OP.bitwise_xor
                )
                yi_s = y.bitcast(i32)
                nc.vector.tensor_scalar(
                    yi_s[:], yi_s[:], 0x5F3759E0, None, op0=OP.add
                )
                u = stats.tile([P, nib], f32, tag="u")
                for _ in range(2):
                    nc.vector.tensor_mul(u[:], y[:], y[:])
                    nc.vector.tensor_mul(u[:], u[:], a[:])
                    nc.vector.tensor_scalar(
                        u[:], u[:], -0.5, 1.5, op0=OP.mult, op1=OP.add
                    )
                    nc.vector.tensor_mul(y[:], y[:], u[:])
                for i in range(nib):
                    ib = ib0 + i
                    gi = c * NIB + ib
                    o = epi.tile([P, D], f32, tag="o")
                    nc.vector.tensor_scalar(
                        o[:],
                        z_tiles[ib][:],
                        mv_b[:, 2 * ib : 2 * ib + 1],
                        y[:, i : i + 1],
                        op0=OP.subtract,
                        op1=OP.mult,
                    )
                    nc.gpsimd.tensor_mul(o[:], o[:], gm_bc[:])
                    nc.gpsimd.tensor_add(o[:], o[:], bt_bc[:])
                    nc.sync.dma_start(out_rows[gi], o[:])

            def score_tile(qTc, jt):
                """scoresT psum tile for key-tile jt vs the chunk's queries,
                exp'd into a bf16 w tile."""
                ps = psA.tile([P, CH], f32, tag="ps_sc")
                if fp8:
                    # DoubleRow: both d-halves contracted in one MM
                    # (lhsT [Ki, 2, M], rhs [Ki, 2, N])
                    for q in range(NQ):
                        nc.tensor.matmul(
                            ps[:, q * 512 : (q + 1) * 512],
                            kT[:, :, jt * P : (jt + 1) * P],
                            qTc[:, :, q * 512 : (q + 1) * 512],
                            start=True,
                            stop=True,
                            perf_mode=mybir.MatmulPerfMode.DoubleRow,
                        )
                else:
                    for kd in range(ND):
                        for q in range(NQ):
                            nc.tensor.matmul(
                                ps[:, q * 512 : (q + 1) * 512],
                                kT[:, kd, jt * P : (jt + 1) * P],
                                qTc[:, kd, q * 512 : (q + 1) * 512],
                                start=(kd == 0),
                                stop=(kd == ND - 1),
                            )
                wt = wpool.tile([P, CH], bf16, tag="w")
                nc.scalar.activation(wt[:], ps[:], AF.Exp, scale=SCALE)
                return wt

            def body():
                # chunk-0 x_t was DMA'd before everything else; its PE-side
                # prep overlaps the x_s DMA stream
                prep = {0: prep_compute(0, (xt0_tiles, xtT0))}

                # ---- phase A: x_s side, software-pipelined: each XB-batch's
                # transposes, kT slice, v slice, and chunk-0 scores for the
                # batch's 4 key-tiles run while later DMAs stream in. ACT is
                # kept exp-only (copies and bias adds on DVE). ----
                nc.vector.memset(v_sb[:, :, D : D + 1], 1.0)
                w_tiles0 = []
                for bt in range(NS // XB):
                    s0 = bt * XB * P  # 512 columns per batch
                    # host-pre-transposed x_s slice streams straight into xsT
                    nc.sync.dma_start(
                        xsT[:, :, s0 : s0 + XB * P],
                        xst_d.ap()[:, :, s0 : s0 + XB * P],
                    )
                    if stages == 10:
                        continue
                    for mo in range(ND):
                        ps = psA.tile([P, 512], f32, tag="ps_sc")
                        for kd in range(ND):
                            nc.tensor.matmul(
                                ps[:],
                                wks[:, kd, mo * P : (mo + 1) * P],
                                xsT[:, kd, s0 : s0 + 512],
                                start=(kd == 0),
                                stop=(kd == ND - 1),
                            )
                        nc.scalar.activation(
                            kT[:, mo, s0 : s0 + 512],
                            ps[:],
                            AF.Identity,
                            bias=bk_sb[:, mo : mo + 1],
                        )
                    for st in range(bt * XB, (bt + 1) * XB):
                        ps = psA.tile([P, D], f32, tag="ps_sc")
                        for kd in range(ND):
                            nc.tensor.matmul(
                                ps[:],
                                xsT[:, kd, st * P : (st + 1) * P],
                                wvs[:, kd, :],
                                start=(kd == 0),
                                stop=(kd == ND - 1),
                            )
                        nc.vector.tensor_add(v_sb[:, st, 0:D], ps[:], bv_bc[:])
                    if stages >= 1 and weave:
                        for jt in range(bt * XB, (bt + 1) * XB):
                            w_tiles0.append(score_tile(prep[0][1], jt))

                if stages == 10:
                    consume_tiles([xsT[:, mo, :] for mo in range(ND)])
                    return

                if stages == 0:
                    consume_tiles(
                        [prep[0][1][:, mo, :] for mo in range(ND)]
                        + [kT[:, mo, :] for mo in range(ND)]
                        + [v_sb[:, jt, :] for jt in range(NS)]
                    )
                    return

                # ---- main loop: attention per query chunk ----
                for c in range(NCH):
                    qTc = prep[c][1]
                    if c == 0 and weave:
                        w_tiles = w_tiles0
                    else:
                        w_tiles = [score_tile(qTc, jt) for jt in range(NS)]

                    # next chunk's x_t DMA streams during this chunk's
                    # combined phase; its PE work is traced after (below)
                    if c + 1 < NCH:
                        next_tiles = prep_load(c + 1)

                    if stages == 1:
                        consume_tiles(w_tiles)
                        if c + 1 < NCH:
                            prep[c + 1] = prep_compute(c + 1, next_tiles)
                        continue

                    den_b = stats.tile([P, NIB], f32, tag="den")
                    mv_b = stats.tile([P, 2 * NIB], f32, tag="mv")
                    z_tiles = []
                    for ib in range(NIB):
                        pc = psB.tile([P, D + 1], f32, tag="ps_small")
                        for jt in range(NS):
                            nc.tensor.matmul(
                                pc[:],
                                w_tiles[jt][:, ib * P : (ib + 1) * P],
                                v_sb[:, jt, :],
                                start=(jt == 0),
                                stop=(jt == NS - 1),
                            )
                        if stages == 2:
                            dst = epi.tile([P, D + 1], f32, tag="z")
                            nc.vector.tensor_copy(dst[:], pc[:])
                            nc.sync.dma_start(
                                out_rows[c * NIB + ib][:, 0:1], dst[:, 0:1]
                            )
                            continue
                        xn_res = prep[c][0][ib // XB][:, ib % XB, :]
                        epilogue_block(pc, ib, xn_res, den_b, mv_b, z_tiles)
                        # drain the tail early: halves normally, pairs on
                        # the final chunk (shortens the post-matmul drain)
                        if stages != 2:
                            if c == NCH - 1 and ib % 2 == 1 and ib < NIB - 1:
                                epilogue_tail(c, mv_b, z_tiles, ib - 1, 2)
                            elif c < NCH - 1 and ib == NIB // 2 - 1:
                                epilogue_tail(c, mv_b, z_tiles, 0, NIB // 2)
                    # next chunk's PE-side prep precedes the epilogue tail so
                    # its qT is ready before scores(c+1) (tail is DVE/Pool)
                    if c + 1 < NCH:
                        prep[c + 1] = prep_compute(c + 1, next_tiles)
                    if stages != 2:
                        if c == NCH - 1:
                            epilogue_tail(c, mv_b, z_tiles, NIB - 2, 2)
                        else:
                            epilogue_tail(c, mv_b, z_tiles, NIB // 2, NIB // 2)

            loop_cm = (
                tc.For_i(0, loop_n, 1) if loop_n > 0 else contextlib.nullcontext()
            )
            with loop_cm:
                for _rep in range(repeat):
                    body()

    nc.compile()
    return nc


def _get_nc(repeat=1, loop_n=0, stages=3, weave=0, fp8=0):
    key = ("nc", repeat, loop_n, stages, weave, fp8)
    if key not in _CACHE:
        _CACHE[key] = _build_nc(repeat, loop_n, stages, weave, fp8)
    return _CACHE[key]


def _make_in_maps(
    supervised_embedding,
    transformer_embedding,
    Wq,
    bq,
    Wk,
    bk,
    Wv,
    bv,
    gamma,
    beta,
):
    bf = ml_dtypes.bfloat16
    f32 = np.float32
    shared = {
        "wqt": np.ascontiguousarray(np.asarray(Wq, f32).T).astype(bf),
        "wkt": np.ascontiguousarray(np.asarray(Wk, f32).T).astype(bf),
        "wvt": np.ascontiguousarray(np.asarray(Wv, f32).T).astype(bf),
        "bq": np.ascontiguousarray(np.asarray(bq, f32)),
        "bk": np.ascontiguousarray(np.asarray(bk, f32)),
        "bv": np.ascontiguousarray(np.asarray(bv, f32)),
        "gamma": np.ascontiguousarray(np.asarray(gamma, f32)),
        "beta": np.ascontiguousarray(np.asarray(beta, f32)),
    }
    xs_all = np.asarray(supervised_embedding, f32)
    xt_all = np.asarray(transformer_embedding, f32)

    def xT(x):
        # [S, D] rows -> [P, ND, S] with xT[p, kd, s] = x[s, kd*P + p]
        # (single-pass cast+copy: astype order='C' materializes the
        # transposed view directly as contiguous bf16)
        return x.reshape(4096, 2, 128).transpose(2, 1, 0).astype(bf, order="C")

    return [
        {
            "xt": np.ascontiguousarray(xt_all[b]),
            "xst": xT(xs_all[b]),
            "xtt": xT(xt_all[b]),
            **shared,
        }
        for b in range(B)
    ]


def kernel(**inputs):
    from concourse.bass_utils import run_bass_kernel_spmd

    nc = _get_nc()
    in_maps = _make_in_maps(**inputs)
    res = run_bass_kernel_spmd(nc, in_maps, core_ids=list(range(B)))
    return np.stack([res.results[b]["out"] for b in range(B)], axis=0)
